# revision 14
# baseline (speedup 1.0000x reference)
"""C3DLoss kernel for Trainium2 — 8-core batch-parallel, raw-Bass, v10.

v9 = baseline v2 pipeline (slot-granular squares/PE/exp, swept plan)
with ONE structural change: the DVE subtract covers BOTH terms of a
shift in a single 6-plane instruction (halves sub instruction count and
fixed access overhead; DVE -16us).  Slot order is re-paired so the two
terms of one shift are adjacent slots (even=term0, odd=term1), which
also merges the per-slab DMA into one q + one r load.

Per core = one batch frame b (pose partner tb = b^1):
    partial = sum over terms t in {same, cross}, shifts delta in
              [-2,2]^2, pixels p of
        exp(-50 * sum_c (ref_c(p) - q_c(p+delta))^2)
    with masks PRE-BAKED into the z channel on the host.
    loss = -(sum of partials) / max(sum(depth_gt_mask), 1)
"""

import os
import sys

for _p in ("/opt/trn_rl_repo", "/opt/pypackages"):
    if _p not in sys.path:
        sys.path.insert(0, _p)

from contextlib import ExitStack

import numpy as np
import ml_dtypes
from numpy.lib.stride_tricks import sliding_window_view

import concourse.bass as bass
import concourse.mybir as mybir
from concourse.ap import AP
from concourse.alu_op_type import AluOpType

PARANOID = os.environ.get("KV9_PARANOID") == "1"

F32 = mybir.dt.float32
BF16 = mybir.dt.bfloat16
BF_NP = ml_dtypes.bfloat16

R = 2
H, W = 352, 1216
G = 64
WB = W // G            # 19
WBH = WB + 2 * R       # 23
Hp = H + 2 * R         # 356
HS = 32
NSLAB = H // HS        # 11
NQP = Hp * WBH         # 8188
QF = (HS + 2 * R) * WBH   # 828
RF = HS * WBH             # 736
SQF = HS * WB             # 608
SLF = 3 * SQF             # 1824  per-slot (3-plane) free size
DFF = 6 * SQF             # 3648  per-job (6-plane) df free size
NCH = (HS // 2) * WB      # 304
BIG = 30.0
EXP_SCALE = -50.0

N_PRE = 8   # slab-0 gb ops emitted before the main loop, overlapping
            # the initial q(planes1-5)+r DMA (they only need q plane 0)
NSQ = 18    # sq ring (slots)
NDF = 6     # df ring (jobs)
LA = 4      # ACT square lookahead (slots)

NSLOT = NSLAB * 50        # 550 (slot k: term=k%2, shift=(k%50)//2)
NJOB = NSLOT // 2         # 275
NUNIT = NSLOT             # psum chunk-units
EUN = 4
NEU = (NUNIT + EUN - 1) // EUN
NACC = 288

SHIFTS = [(dy, dx) for dy in range(-R, R + 1) for dx in range(-R, R + 1)]
# term0 rgb symmetry: ref_rgb == query_rgb == rgb_b, so the (g,b)-plane
# squares of shift -d are a shifted copy of those of +d.  Shifts i<12
# ("producers") compute a haloed gb square tile; i>12 ("mirrors") reuse
# it; i==12 (d=0) is normal.  Mirror of i is 24-i.
GB_EXT = [(HS + abs(dy), WB + abs(dx)) for (dy, dx) in SHIFTS]
GB_TS = max(er * ec for (er, ec) in GB_EXT[:12])   # 714 max tile free size


def gb_geom(it):
    """Producer shift it<12: tile extent + read offsets.

    Returns (er, ec, prod_off, mirr_off): row/col extent, and the
    (row, col) offsets at which the producer slot (shift it) and the
    mirror slot (shift 24-it) read their HSxWB windows."""
    dy, dx = SHIFTS[it]
    er, ec = HS + abs(dy), WB + abs(dx)
    prod = (0, max(dx, 0))
    mirr = (-dy, max(-dx, 0))
    return er, ec, prod, mirr

# Square assignment per slot (3 planes), same machinery as v2 but with
# the plan re-swept for the lower DVE load: D:A:P = 16:35:21 per 72.
_A3 = [("A", 0, 3)]
_P3 = [("P", 0, 3)]
_D3 = [("D", 0, 3)]
_PD = [("P", 0, 2), ("D", 2, 1)]
_AD = [("A", 0, 2), ("D", 2, 1)]
_DP = [("D", 0, 2), ("P", 2, 1)]
_DA = [("D", 0, 1), ("A", 1, 2)]
_AA = [("A", 1, 5)]       # merged: even slot planes 1-2 + odd slot 0-2
_AX = [("AX", 0, 0)]      # marker: covered by previous slot's _AA op
# 24-slot cycle, re-swept for v10: the gb-halo ops load DVE, so its
# square share drops (first half's _D3 -> _DA); D:A:P = 11:38:23 per 72.
# three merged ACT pairs (_AA/_AX) per 24 slots; loads rebalanced via
# the _D3/_P3 swaps in the second half
_CYC24 = ([_A3, _P3, _AA, _AX, _PD, _A3, _P3, _A3, _A3, _P3, _AA, _AX]
          + [_A3, _P3, _D3, _A3, _PD, _P3, _AA, _AX, _AD, _P3, _A3, _DP])
# 50-slot slab-aligned cycle: every slab sees the identical pattern
_CYC = _CYC24 + _CYC24 + [_A3, _P3]
PLAN = []
SQLIST = {}
ORD2 = {}


def _sym_slot(k):
    # every term0 slot skips its gb plane (local 0) in the df/sq path:
    # non-center shifts use the shared haloed gb tile instead, and the
    # center shift's gb diff is identically zero (same image, no shift)
    return k % 2 == 0


TAILN = 8   # last slots' squares forced onto DVE: shortens the drain
            # (no cross-engine hop) and clears ACT's queue for final exps


def set_plan(cyc):
    global PLAN, SQLIST, ORD2
    PLAN = []
    for k in range(NSLOT):
        ops = cyc[k % len(cyc)]
        if k >= NSLOT - TAILN:
            prev = cyc[(k - 1) % len(cyc)]
            own_ax = any(e == "AX" for (e, _l, _n) in ops)
            spills = any(e == "A" and n > 3 for (e, _l, n) in prev)
            if not (own_ax and spills) and not any(
                    e == "A" and n > 3 for (e, _l, n) in ops):
                PLAN.append([("D", 1, 2)] if k % 2 == 0 else [("D", 0, 3)])
                continue
        if _sym_slot(k) and ops[0][0] not in ("AX",) and ops != _AA:
            clipped = []
            for (e, lo, np_) in ops:
                nlo = max(lo, 1)
                hi = lo + np_
                if hi > nlo:
                    clipped.append((e, nlo, hi - nlo))
            ops = clipped
        PLAN.append(ops)
    SQLIST = {"D": [], "A": [], "P": []}
    ORD2 = {}
    for k in range(NSLOT):
        for (e, lo, np_) in PLAN[k]:
            if e == "AX":
                ORD2[(k, "A")] = ORD2[(k - 1, "A")]
                continue
            ORD2[(k, e)] = len(SQLIST[e])
            SQLIST[e].append((k, lo, np_))
    _CACHED.clear()


_CACHED = {}
set_plan(_CYC)


def _apv(t_ap, p0, pcnt, free_dims, free_off=0):
    pstride = t_ap.ap[0][0]
    base = t_ap.offset + p0 * pstride + free_off
    return AP(t_ap.tensor, base, [[pstride, pcnt]] + [list(d) for d in free_dims])


def _dram_ap(handle, offset, dims):
    a = handle[:]
    return AP(a.tensor, a.offset + offset, [list(d) for d in dims])


def make_sel():
    s = np.zeros((128, G), dtype=BF_NP)
    for c in range(2):
        for g in range(G):
            s[c * G + g, g] = 1
    return s


def emit(nc: bass.Bass):
    dp = nc.declare_dram_parameter
    q_d = dp("q_d", [128, 6 * NQP], BF16, isOutput=False)
    r_d = dp("r_d", [128, 6 * NQP], BF16, isOutput=False)
    sel_d = dp("sel_d", [128, G], BF16, isOutput=False)
    res_d = dp("res_d", [128, 1], F32, isOutput=True)

    with ExitStack() as ex:
        E = ex.enter_context
        qt = [E(nc.sbuf_tensor(f"qt{p}", [128, 6 * QF], BF16))
              for p in range(2)]
        rt = [E(nc.sbuf_tensor(f"rt{p}", [128, 6 * RF], BF16))
              for p in range(2)]
        df = E(nc.sbuf_tensor("df", [128, NDF * DFF], BF16))
        gb2 = E(nc.sbuf_tensor("gb2", [128, 2 * 12 * GB_TS], BF16))
        sq = E(nc.sbuf_tensor("sq", [128, NSQ * SLF], BF16))
        sel_s = E(nc.sbuf_tensor("sel", [128, G], BF16))
        acc_s = E(nc.sbuf_tensor("acc", [128, NACC], F32))
        res_s = E(nc.sbuf_tensor("res", [128, 1], F32))
        pst = E(nc.psum_tensor("pst", [128, 4096], F32))

        sLC = E(nc.semaphore("sLC"))
        sLt = [E(nc.semaphore(f"sL{p}")) for p in range(2)]
        sV = E(nc.semaphore("sV"))    # DVE subs, 1/JOB (+ final reduce)
        sQD = E(nc.semaphore("sQD"))
        sQA = E(nc.semaphore("sQA"))
        sQP = E(nc.semaphore("sQP"))
        sP = E(nc.semaphore("sP"))    # PE: 2 per slot (per chunk)
        sA = E(nc.semaphore("sA"))    # ACT exp EUs
        sGB = E(nc.semaphore("sGB"))  # gb halo squares, 1/(slab,prod-shift)
        sQ0 = E(nc.semaphore("sQ0"))  # slab-0 q plane-0 early load
        sGS = E(nc.semaphore("sGS"))  # prefix gb diff pairs (DVE -> ACT)
        sGBA = E(nc.semaphore("sGBA"))  # prefix gb squares done (on ACT)
        blk = E(nc.Block())

        semQ = {"D": sQD, "A": sQA, "P": sQP}

        def df_ap(k, lo, np_):
            # slot k's 3-plane half inside job tile (k//2)%NDF
            off = ((k // 2) % NDF) * DFF + (k % 2) * SLF + lo * SQF
            return _apv(df.ap(), 0, 128, [[1, np_ * SQF]], off)

        def sub_wait(eng, k):
            eng.wait_ge(sV, k // 2 + 1)

        def sq_ring_wait(eng, k):
            if k >= NSQ:
                eng.wait_ge(sP, 2 * (k - NSQ) + 2)

        def sq_done_wait(eng, k):
            for (e, _lo, _np) in PLAN[k]:
                e = "A" if e == "AX" else e
                eng.wait_ge(semQ[e], ORD2[(k, e)] + 1)

        @blk.gpsimd
        def _(gp):
            gp.memset(acc_s.ap(), 0.0)
            gp.drain()
            gp.sem_inc(sLC, 1)
            for (k, lo, np_) in SQLIST["P"]:
                gp.wait_ge(sV, k // 2 + 1)
                sq_ring_wait(gp, k)
                gp.tensor_mul(
                    _apv(sq.ap(), 0, 128, [[1, np_ * SQF]],
                         (k % NSQ) * SLF + lo * SQF),
                    df_ap(k, lo, np_),
                    df_ap(k, lo, np_),
                ).then_inc(sQP, 1)

        @blk.sync
        def _(sp):
            sp.dma_start(sel_s[:], sel_d[:]).then_inc(sLC, 16)
            for s in range(NSLAB):
                ph = s % 2
                if s >= 2:
                    sp.wait_ge(sV, 25 * (s - 1))
                r0 = s * HS
                if s == 0:
                    sp.dma_start(
                        _apv(qt[0].ap(), 0, 128, [[1, QF]]),
                        _dram_ap(q_d, 0, [[6 * NQP, 128], [1, QF]])
                    ).then_inc(sQ0, 16)
                    sp.dma_start(
                        _apv(qt[0].ap(), 0, 128, [[QF, 5], [1, QF]], QF),
                        _dram_ap(q_d, NQP,
                                 [[6 * NQP, 128], [NQP, 5], [1, QF]])
                    ).then_inc(sLt[0], 16)
                else:
                    sp.dma_start(
                        qt[ph].ap(),
                        _dram_ap(q_d, r0 * WBH,
                                 [[6 * NQP, 128], [NQP, 6], [1, QF]])
                    ).then_inc(sLt[ph], 16)
                sp.dma_start(
                    rt[ph].ap(),
                    _dram_ap(r_d, (r0 + 2) * WBH,
                             [[6 * NQP, 128], [NQP, 6], [1, RF]])
                ).then_inc(sLt[ph], 16)
            sp.wait_ge(sV, NJOB + 1)
            sp.dma_start(res_d[:], res_s.ap()).then_inc(sLC, 16)

        def emit_gb(ve, s, ph, i):
            # diff for producer shift i; on odd i also square the pair
            # (i-1, i) in one in-place op over both adjacent tiles
            dy, dx = SHIFTS[i]
            qoff = (2 + dy) * WBH + (2 + dx)
            er, ec, _prod, _mirr = gb_geom(i)
            goff = (ph * 12 + i) * GB_TS
            if s >= 2:
                ve.wait_ge(sP, 2 * 50 * (s - 1))
            gap = _apv(gb2.ap(), 0, 128, [[ec, er], [1, ec]], goff)
            cb = -max(dx, 0)
            nc.vector.tensor_tensor(
                gap,
                _apv(qt[ph].ap(), 0, 128, [[WBH, er], [1, ec]],
                     2 * WBH + 2 + cb),
                _apv(qt[ph].ap(), 0, 128, [[WBH, er], [1, ec]],
                     qoff + cb),
                AluOpType.subtract)
            if i % 2 == 1:
                if s == 0 and i < N_PRE:
                    # square done by ACT (top of its stream); signal diffs
                    ve.sem_inc(sGS, 1)
                else:
                    g2 = _apv(gb2.ap(), 0, 128, [[1, GB_TS + er * ec]],
                              (ph * 12 + i - 1) * GB_TS)
                    nc.vector.tensor_mul(g2, g2, g2).then_inc(sGB, 1)

        @blk.vector
        def _(ve):
            ve.wait_ge(sLC, 16)
            ve.wait_ge(sQ0, 16)
            for i0 in range(min(N_PRE, 12)):
                emit_gb(ve, 0, 0, i0)
            for k in range(NSLOT):
                s = k // 50
                ph = s % 2
                if k % 50 == 0:
                    ve.wait_ge(sLt[ph], 32 * (s // 2 + 1))
                if k % 2 == 0:
                    # one multi-plane sub covers both terms of this shift
                    j = k // 2
                    i = j % 25
                    dy, dx = SHIFTS[i]
                    qoff = (2 + dy) * WBH + (2 + dx)
                    if j >= NDF:
                        sq_done_wait(ve, 2 * (j - NDF))
                        sq_done_wait(ve, 2 * (j - NDF) + 1)
                    npl, plo = 5, 1   # skip term0 gb (plane 0)
                    nc.vector.tensor_tensor(
                        _apv(df.ap(), 0, 128,
                             [[SQF, npl], [WB, HS], [1, WB]],
                             (j % NDF) * DFF + plo * SQF),
                        _apv(rt[ph].ap(), 0, 128,
                             [[RF, npl], [WBH, HS], [1, WB]],
                             plo * RF + 2),
                        _apv(qt[ph].ap(), 0, 128,
                             [[QF, npl], [WBH, HS], [1, WB]],
                             plo * QF + qoff),
                        AluOpType.subtract).then_inc(sV, 1)
                    if i < 12 and not (s == 0 and i < N_PRE):
                        # producer: haloed gb diff + square into gb2[ph][i]
                        emit_gb(ve, s, ph, i)
                for (e, lo, np_) in PLAN[k]:
                    if e != "D":
                        continue
                    sq_ring_wait(ve, k)
                    if PARANOID:
                        ve.wait_ge(sV, k // 2 + 1)
                    nc.vector.tensor_mul(
                        _apv(sq.ap(), 0, 128, [[1, np_ * SQF]],
                             (k % NSQ) * SLF + lo * SQF),
                        df_ap(k, lo, np_),
                        df_ap(k, lo, np_),
                    ).then_inc(sQD, 1)
            ve.wait_ge(sA, NEU)
            nc.vector.tensor_reduce(
                res_s.ap(), acc_s.ap(), axis=mybir.AxisListType.X,
                op=AluOpType.add).then_inc(sV, 1)

        @blk.tensor
        def _(pe):
            pe.wait_ge(sLC, 16)
            for k in range(NSLOT):
                par = k % 2
                pr = k // 2
                if par == 0 and 2 * pr >= 8:
                    pe.wait_ge(sA, (2 * pr - 8) // EUN + 1)
                sq_done_wait(pe, k)
                s_ = k // 50
                ph_ = s_ % 2
                i_ = (k % 50) // 2
                gbt = None
                skip0 = (par == 0 and i_ == 12)
                if par == 0 and i_ != 12:
                    it = i_ if i_ < 12 else 24 - i_
                    er, ec, prod, mirr = gb_geom(it)
                    ro, co = prod if i_ < 12 else mirr
                    goff = (ph_ * 12 + it) * GB_TS
                    if s_ == 0 and it < N_PRE:
                        pe.wait_ge(sGBA, it // 2 + 1)
                    else:
                        pe.wait_ge(sGB, s_ * 6 + it // 2 + 1 - N_PRE // 2)
                    gbt = (er, ec, ro, co, goff)
                for c in range(2):
                    u = 2 * pr + c
                    col = 512 * (u % 8)
                    for t in range(3):
                        if t == 0 and skip0:
                            continue
                        if t == 0 and gbt is not None:
                            er, ec, ro, co, goff = gbt
                            rhs = _apv(gb2.ap(), 0, 128,
                                       [[ec, HS // 2], [1, WB]],
                                       goff + (ro + (HS // 2) * c) * ec + co)
                        else:
                            rhs = _apv(sq.ap(), 0, 128, [[1, NCH]],
                                       (k % NSQ) * SLF + t * SQF + NCH * c)
                        mm = nc.tensor.matmul(
                            pst[G * par:G * par + G, col:col + NCH], sel_s[:],
                            rhs,
                            start=(t == 1 if skip0 else t == 0),
                            stop=(t == 2),
                            skip_group_check=True,
                            tile_position=(0, G * par))
                        if t == 2:
                            mm.then_inc(sP, 1)

        @blk.scalar
        def _(ac):
            ac.wait_ge(sLC, 17)
            for p_ in range(N_PRE // 2):
                i1 = 2 * p_ + 1
                er1, ec1, _pr, _mi = gb_geom(i1)
                ac.wait_ge(sGS, p_ + 1)
                g2 = _apv(gb2.ap(), 0, 128, [[1, GB_TS + er1 * ec1]],
                          2 * p_ * GB_TS)
                nc.scalar.activation(
                    g2, g2,
                    mybir.ActivationFunctionType.Square).then_inc(sGBA, 1)
            aptr = 0

            act_list = SQLIST["A"]

            def flush(upto_slot, aptr):
                while aptr < len(act_list) and act_list[aptr][0] < upto_slot:
                    k, lo, np_ = act_list[aptr]
                    ac.wait_ge(sV, k // 2 + 1)
                    sq_ring_wait(ac, k)
                    ks = (k % NSQ) * SLF + lo * SQF
                    if np_ == 5:
                        # merged pair: spills into slot k+1's tile region
                        sq_ring_wait(ac, k + 1)
                    nc.scalar.activation(
                        _apv(sq.ap(), 0, 128, [[1, np_ * SQF]], ks),
                        df_ap(k, lo, np_),
                        mybir.ActivationFunctionType.Square).then_inc(sQA, 1)
                    aptr += 1
                return aptr

            for e in range(NEU):
                u0 = EUN * e
                nu = min(EUN, NUNIT - u0)
                last_slot = 2 * ((u0 + nu - 1) // 2) + 1
                aptr = flush(min(last_slot + 1 + LA, NSLOT), aptr)
                ac.wait_ge(sP, 2 * (2 * ((u0 + nu - 1) // 2) + 2))
                col = 512 * (u0 % 8)
                ap_io = AP(pst[:].tensor, pst[:].offset + col,
                           [[pst[:].ap[0][0], 128], [512, nu], [1, NCH]])
                nc.scalar.activation(
                    ap_io, ap_io,
                    mybir.ActivationFunctionType.Exp,
                    scale=EXP_SCALE,
                    accum_out=acc_s[:, e:e + 1]).then_inc(sA, 1)
            flush(NSLOT, aptr)
    return nc


# ---------------- host side ----------------

def _pad(x, fill=0.0):
    p = np.full((Hp, W + 2 * R), fill, np.float32)
    p[R:R + H, R:R + W] = x
    return p


def _block_tiles(planes):
    flat = []
    for c0, c1 in planes:
        flat.append(c0)
        flat.append(c1)
    P = np.stack(flat)                                     # [12, Hp, Wp]
    sw = sliding_window_view(P, WBH, axis=2)[:, :, ::WB]   # [12, Hp, G, WBH]
    blocked = np.ascontiguousarray(sw.transpose(0, 2, 1, 3))
    b = blocked.reshape(6, 2, G, NQP).transpose(1, 2, 0, 3)
    return b.astype(BF_NP).reshape(128, 6 * NQP)


def host_precompute(rgb, depth, depth_gt, depth_mask, depth_gt_mask,
                    xy1_grid, Ts, b):
    tb = b ^ 1
    f32 = np.float32
    xy1 = np.asarray(xy1_grid[b], f32)
    dep = np.asarray(depth[b, 0], f32)
    dgt_b = np.asarray(depth_gt[b, 0], f32)
    dgt_t = np.asarray(depth_gt[tb, 0], f32)
    mp = np.asarray(depth_mask[b, 0], f32)
    mg_b = np.asarray(depth_gt_mask[b, 0], f32)
    mg_t = np.asarray(depth_gt_mask[tb, 0], f32)
    rgb_b = np.asarray(rgb[b], f32)
    rgb_t = np.asarray(rgb[tb], f32)

    xyz_p = xy1 * dep
    xyz_gb = xy1 * dgt_b
    xyz_gt = xy1 * dgt_t
    T21 = (np.linalg.inv(np.asarray(Ts[tb], np.float64)) @
           np.asarray(Ts[b], np.float64)).astype(f32)
    Rm, tv = T21[:3, :3], T21[:3, 3]
    txyz = np.einsum('ij,jhw->ihw', Rm, xyz_p).astype(f32) \
        + tv[:, None, None].astype(f32)
    posq = (txyz[2] > 0).astype(f32) * mp

    qz0 = xyz_p[2] + BIG * (1.0 - mp)
    qz1 = txyz[2] + BIG * (1.0 - posq)
    rz0 = xyz_gb[2] - BIG * (1.0 - mg_b)
    rz1 = xyz_gt[2] - BIG * (1.0 - mg_t)

    p_rb = [_pad(rgb_b[0]), _pad(rgb_b[1]), _pad(rgb_b[2])]
    p_rt = [_pad(rgb_t[0]), _pad(rgb_t[1]), _pad(rgb_t[2])]
    q = _block_tiles([
        (p_rb[1], p_rb[2]),
        (_pad(xyz_p[0]), _pad(xyz_p[1])),
        (_pad(qz0, BIG), p_rb[0]),
        (_pad(txyz[0]), _pad(txyz[1])),
        (_pad(qz1, BIG), p_rb[0]),
        (p_rb[1], p_rb[2]),
    ])
    r = _block_tiles([
        (p_rb[1], p_rb[2]),
        (_pad(xyz_gb[0]), _pad(xyz_gb[1])),
        (_pad(rz0), p_rb[0]),
        (_pad(xyz_gt[0]), _pad(xyz_gt[1])),
        (_pad(rz1), p_rt[0]),
        (p_rt[1], p_rt[2]),
    ])
    return {"q_d": q, "r_d": r, "sel_d": make_sel()}


def make_in_maps(rgb, depth, depth_gt, depth_mask, depth_gt_mask, xy1_grid, Ts,
                 n_cores=8):
    return [host_precompute(rgb, depth, depth_gt, depth_mask, depth_gt_mask,
                            xy1_grid, Ts, b) for b in range(n_cores)]


def _get_nc():
    if "nc" not in _CACHED:
        nc = bass.Bass()
        emit(nc)
        _CACHED["nc"] = nc
    return _CACHED["nc"]


def kernel(rgb, depth, depth_gt, depth_mask, depth_gt_mask, xy1_grid, Ts,
           **run_kwargs):
    from concourse.bass_utils import run_bass_kernel_spmd
    nc = _get_nc()
    maps = make_in_maps(rgb, depth, depth_gt, depth_mask, depth_gt_mask,
                        xy1_grid, Ts)
    res = run_bass_kernel_spmd(nc, maps, list(range(8)), **run_kwargs)
    total = np.float64(0.0)
    for r in res.results:
        total += np.float64(r["res_d"][:, 0].sum())
    n_gt = max(np.asarray(depth_gt_mask, np.float64).sum(), 1.0)
    loss = -total / n_gt
    kernel.last_results = res
    return np.float32(loss)
